# revision 4
# baseline (speedup 1.0000x reference)
"""Performer (FAVOR+) attention on 8 trn2 NeuronCores.

The axon tunnel serializes transfers per client connection (~40-60MB/s,
~80ms/call RTT), but separate OS processes get independent connections
with near-linear aggregate scaling. So the fast path runs a fleet of 8
worker processes, each owning one NeuronCore and one shard of the
computation (batch b = wid//2, head-group hg = wid%2 -> 8 of 16 heads):

  - inputs are cast to bf16 once on the host, shipped to workers through
    shared memory, and kept device-resident across calls (reuse is
    verified bit-exactly against cached host copies),
  - each worker computes its [S, 512] output slice on its core (bf16
    matmuls, fp32 accumulation; measured rel err ~0.3%, tolerance 2e-2),
  - the output returns as int8 + per-(head,row) fp16 scales (~2.1MB per
    worker, fetched in parallel across the 8 connections), and the main
    process dequantizes straight into the final fp32 array.

Any failure falls back to a single-process fp32 GSPMD jit (the original
baseline path).
"""
import os
import subprocess
import sys
import threading
import time
import numpy as np
from multiprocessing import shared_memory

B, S, D = 4, 4096, 1024
H = 16
HD = 64
M = 256
N_CORES = 8
HPC = 8                      # heads per worker
COLS = HPC * HD              # 512 output columns per worker

IN_NAMES = ("X", "mask", "Wq", "bq", "Wk", "bk", "Wv", "bv", "proj")

# ---- shared memory layouts -------------------------------------------------
_X_BYTES = B * S * D * 2                      # bf16
_W_BYTES = D * D * 2                          # bf16, each of Wq/Wk/Wv
_MASK_BYTES = B * S * 4
_BIAS_BYTES = D * 4                           # each of bq/bk/bv
_PROJ_BYTES = M * HD * 4
_IN_BYTES = _X_BYTES + 3 * _W_BYTES + _MASK_BYTES + 3 * _BIAS_BYTES + _PROJ_BYTES

_Q_BYTES = S * HPC * HD                       # int8 [S, 8, 64] per worker
_SC_BYTES = S * HPC * 2                       # fp16 [S, 8] per worker
_OUT_STRIDE = _Q_BYTES + _SC_BYTES
_OUT_BYTES = N_CORES * _OUT_STRIDE

_WORKER_SRC = r'''
import sys, os, numpy as np
from multiprocessing import shared_memory

WID = int(sys.argv[1])
SHM_IN, SHM_OUT = sys.argv[2], sys.argv[3]
B, S, D, H, HD, M = 4, 4096, 1024, 16, 64, 256
HPC, COLS = 8, 512
bI = WID // 2
hg = WID % 2

import ml_dtypes
import jax, jax.numpy as jnp
bf16 = ml_dtypes.bfloat16

dev = jax.devices()[WID]

shm_in = shared_memory.SharedMemory(name=SHM_IN)
shm_out = shared_memory.SharedMemory(name=SHM_OUT)

XB, WB, MKB, BB, PB = B*S*D*2, D*D*2, B*S*4, D*4, M*HD*4
o = 0
Xv = np.ndarray((B, S, D), bf16, buffer=shm_in.buf, offset=o); o += XB
Wqv = np.ndarray((D, D), bf16, buffer=shm_in.buf, offset=o); o += WB
Wkv = np.ndarray((D, D), bf16, buffer=shm_in.buf, offset=o); o += WB
Wvv = np.ndarray((D, D), bf16, buffer=shm_in.buf, offset=o); o += WB
maskv = np.ndarray((B, S), np.float32, buffer=shm_in.buf, offset=o); o += MKB
bqv = np.ndarray((D,), np.float32, buffer=shm_in.buf, offset=o); o += BB
bkv = np.ndarray((D,), np.float32, buffer=shm_in.buf, offset=o); o += BB
bvv = np.ndarray((D,), np.float32, buffer=shm_in.buf, offset=o); o += BB
projv = np.ndarray((M, HD), np.float32, buffer=shm_in.buf, offset=o)

QSTRIDE = S*HPC*HD + S*HPC*2
qout = np.ndarray((S, HPC, HD), np.int8, buffer=shm_out.buf,
                  offset=WID*QSTRIDE)
scout = np.ndarray((S, HPC), np.float16, buffer=shm_out.buf,
                   offset=WID*QSTRIDE + S*HPC*HD)

ratio = M ** -0.5
qk_scale = HD ** -0.5        # two HD**-0.25 factors folded together


def compute(Xb, mb, wq, bqs, wk, bks, wv, bvs, projf):
    f32 = jnp.float32
    mm = lambda a, b: jnp.matmul(a, b, preferred_element_type=f32)
    Q = mm(Xb, wq) + bqs                     # [S, 512] f32
    K = mm(Xb, wk) + bks
    V = mm(Xb, wv) + bvs
    m1 = mb[:, None]
    xq = (Q * qk_scale).reshape(S, HPC, HD).transpose(1, 0, 2)       # [8,S,64]
    xk = (K * qk_scale * m1).reshape(S, HPC, HD).transpose(1, 0, 2)
    v = (V * m1).reshape(S, HPC, HD).transpose(1, 0, 2)
    pT = projf.astype(jnp.bfloat16)
    uq = jnp.einsum('hsd,md->hsm', xq.astype(jnp.bfloat16), pT,
                    preferred_element_type=f32)
    uk = jnp.einsum('hsd,md->hsm', xk.astype(jnp.bfloat16), pT,
                    preferred_element_type=f32)
    dq = 0.5 * jnp.sum(xq * xq, axis=-1, keepdims=True)
    dk = 0.5 * jnp.sum(xk * xk, axis=-1, keepdims=True)
    sq = jnp.max(uq, axis=-1, keepdims=True)
    sk = jnp.max(uk, axis=(-1, -2), keepdims=True)
    qp = ratio * (jnp.exp(uq - dq - sq) + 1e-4)
    kp = ratio * (jnp.exp(uk - dk - sk) + 1e-4)
    qpb = qp.astype(jnp.bfloat16)
    kpb = kp.astype(jnp.bfloat16)
    kv = jnp.einsum('hsm,hsd->hmd', kpb, v.astype(jnp.bfloat16),
                    preferred_element_type=f32)
    ksum = jnp.sum(kpb.astype(f32), axis=1)
    z = 1.0 / (jnp.einsum('hsm,hm->hs', qpb, ksum.astype(jnp.bfloat16),
                          preferred_element_type=f32) + 1e-6)
    out = jnp.einsum('hsm,hmd->hsd', qpb, kv.astype(jnp.bfloat16),
                     preferred_element_type=f32) * z[..., None]
    a = jnp.max(jnp.abs(out), axis=-1, keepdims=True)
    sc = jnp.maximum(a, 1e-30) * (1.0 / 127.0)
    q8 = jnp.clip(jnp.round(out / sc), -127, 127).astype(jnp.int8)
    # [8,S,64] -> [S,8,64] so the host can dequantize without a transpose
    return q8.transpose(1, 0, 2), sc[..., 0].T.astype(jnp.float16)


jfn = jax.jit(compute, device=dev)
_dev_args = None


def load_inputs():
    global _dev_args
    args = (
        np.asarray(Xv[bI]),
        np.asarray(maskv[bI]),
        np.asarray(Wqv[:, hg*COLS:(hg+1)*COLS]),
        np.asarray(bqv[hg*COLS:(hg+1)*COLS]),
        np.asarray(Wkv[:, hg*COLS:(hg+1)*COLS]),
        np.asarray(bkv[hg*COLS:(hg+1)*COLS]),
        np.asarray(Wvv[:, hg*COLS:(hg+1)*COLS]),
        np.asarray(bvv[hg*COLS:(hg+1)*COLS]),
        np.asarray(projv),
    )
    _dev_args = tuple(jax.device_put(a, dev) for a in args)
    for a in _dev_args:
        a.block_until_ready()


def run():
    q8, sc = jfn(*_dev_args)
    qout[...] = np.asarray(q8)
    scout[...] = np.asarray(sc)


try:
    load_inputs()
    run()
    print("READY", flush=True)
except Exception as e:
    print("ERR " + repr(e)[:200], flush=True)

for line in sys.stdin:
    cmd = line.strip().split()
    if not cmd:
        continue
    try:
        if cmd[0] == "RUN":
            run()
            print("DONE " + cmd[1], flush=True)
        elif cmd[0] == "LOAD":
            load_inputs()
            run()
            print("DONE " + cmd[1], flush=True)
        elif cmd[0] == "QUIT":
            break
        else:
            print("ERR unknown cmd", flush=True)
    except Exception as e:
        print("ERR " + repr(e)[:200], flush=True)
'''


class _Fleet:
    def __init__(self):
        self.shm_in = shared_memory.SharedMemory(create=True, size=_IN_BYTES)
        self.shm_out = shared_memory.SharedMemory(create=True, size=_OUT_BYTES)
        self.procs = []
        self.host = None
        self.seq = 0
        self._views()

    def _views(self):
        buf = self.shm_in.buf
        import ml_dtypes
        bf16 = ml_dtypes.bfloat16
        o = 0
        self.Xv = np.ndarray((B, S, D), bf16, buffer=buf, offset=o); o += _X_BYTES
        self.Wqv = np.ndarray((D, D), bf16, buffer=buf, offset=o); o += _W_BYTES
        self.Wkv = np.ndarray((D, D), bf16, buffer=buf, offset=o); o += _W_BYTES
        self.Wvv = np.ndarray((D, D), bf16, buffer=buf, offset=o); o += _W_BYTES
        self.maskv = np.ndarray((B, S), np.float32, buffer=buf, offset=o); o += _MASK_BYTES
        self.bqv = np.ndarray((D,), np.float32, buffer=buf, offset=o); o += _BIAS_BYTES
        self.bkv = np.ndarray((D,), np.float32, buffer=buf, offset=o); o += _BIAS_BYTES
        self.bvv = np.ndarray((D,), np.float32, buffer=buf, offset=o); o += _BIAS_BYTES
        self.projv = np.ndarray((M, HD), np.float32, buffer=buf, offset=o)
        ob = self.shm_out.buf
        self.qv = [np.ndarray((S, HPC, HD), np.int8, buffer=ob,
                              offset=c * _OUT_STRIDE) for c in range(N_CORES)]
        self.scv = [np.ndarray((S, HPC), np.float16, buffer=ob,
                               offset=c * _OUT_STRIDE + _Q_BYTES)
                    for c in range(N_CORES)]

    def write_inputs(self, inputs):
        a32 = {n: np.asarray(inputs[n], np.float32) for n in IN_NAMES}
        self.Xv[...] = a32["X"]
        self.Wqv[...] = a32["Wq"]
        self.Wkv[...] = a32["Wk"]
        self.Wvv[...] = a32["Wv"]
        self.maskv[...] = a32["mask"]
        self.bqv[...] = a32["bq"]
        self.bkv[...] = a32["bk"]
        self.bvv[...] = a32["bv"]
        self.projv[...] = a32["proj"]
        self.host = {n: a32[n].copy() for n in IN_NAMES}

    def spawn(self):
        def start(wid):
            return subprocess.Popen(
                [sys.executable, "-c", _WORKER_SRC, str(wid),
                 self.shm_in.name, self.shm_out.name],
                stdin=subprocess.PIPE, stdout=subprocess.PIPE,
                stderr=subprocess.DEVNULL, text=True, bufsize=1,
            )
        # worker 0 first: it populates the persistent neuron compile cache,
        # the rest then compile-hit and come up quickly
        self.procs = [start(0)]
        self._wait_line(0, "READY", timeout=1800)
        for wid in range(1, N_CORES):
            self.procs.append(start(wid))
        for wid in range(1, N_CORES):
            self._wait_line(wid, "READY", timeout=1800)

    def _wait_line(self, wid, expect, timeout):
        # jax/axon write progress noise to stdout; skip until a protocol line
        p = self.procs[wid]
        deadline = time.monotonic() + timeout
        while True:
            result = {}

            def rd():
                result["line"] = p.stdout.readline()

            t = threading.Thread(target=rd, daemon=True)
            t.start()
            t.join(max(0.0, deadline - time.monotonic()))
            line = result.get("line")
            if not line:
                raise RuntimeError(
                    f"worker {wid}: timeout/EOF waiting for {expect!r} "
                    f"(alive={p.poll() is None})")
            if line.startswith(expect):
                return
            if line.startswith("ERR"):
                raise RuntimeError(f"worker {wid}: {line.strip()}")

    def inputs_match(self, inputs):
        if self.host is None:
            return False
        return all(np.array_equal(np.asarray(inputs[n], np.float32),
                                  self.host[n]) for n in IN_NAMES)

    def run(self, reload):
        self.seq += 1
        cmd = ("LOAD " if reload else "RUN ") + str(self.seq) + "\n"
        for p in self.procs:
            p.stdin.write(cmd)
            p.stdin.flush()
        out = np.empty((B, S, D), np.float32)
        done = [False] * N_CORES
        for wid in range(N_CORES):
            self._wait_line(wid, f"DONE {self.seq}",
                            timeout=600 if reload else 120)
            done[wid] = True
            b, hg = wid // 2, wid % 2
            view = out[b, :, hg * COLS:(hg + 1) * COLS].reshape(S, HPC, HD)
            np.multiply(self.qv[wid], self.scv[wid][:, :, None],
                        out=view, casting="unsafe")
        return out

    def close(self):
        for p in self.procs:
            try:
                p.kill()
            except Exception:
                pass
        for shm in (self.shm_in, self.shm_out):
            try:
                shm.close()
                shm.unlink()
            except Exception:
                pass


_fleet = None
_use_fast = True
_lock = threading.Lock()


def _run_fast(inputs):
    global _fleet
    if _fleet is None:
        f = _Fleet()
        try:
            f.write_inputs(inputs)
            f.spawn()
        except Exception:
            f.close()
            raise
        _fleet = f
        out = np.empty((B, S, D), np.float32)
        for wid in range(N_CORES):
            b, hg = wid // 2, wid % 2
            view = out[b, :, hg * COLS:(hg + 1) * COLS].reshape(S, HPC, HD)
            np.multiply(f.qv[wid], f.scv[wid][:, :, None],
                        out=view, casting="unsafe")
        return out
    if _fleet.inputs_match(inputs):
        return _fleet.run(reload=False)
    _fleet.write_inputs(inputs)
    return _fleet.run(reload=True)


# ----- fallback: plain fp32 GSPMD on the in-process jax client -----

def _run_fallback(X, mask, Wq, bq, Wk, bk, Wv, bv, proj):
    import jax
    import jax.numpy as jnp
    from jax.sharding import Mesh, NamedSharding, PartitionSpec as P

    devs = jax.devices()[:N_CORES]
    mesh = Mesh(np.array(devs), ('x',))
    rep = NamedSharding(mesh, P())
    col = NamedSharding(mesh, P(None, 'x'))
    vec = NamedSharding(mesh, P('x'))
    seq = NamedSharding(mesh, P(None, 'x', None))
    outsh = NamedSharding(mesh, P(None, None, 'x'))

    def compute(X, mask, Wq, bq, Wk, bk, Wv, bv, proj):
        def split(x):
            return x.reshape(B, S, H, HD).transpose(0, 2, 1, 3)
        Q = split(X @ Wq + bq)
        K = split(X @ Wk + bk)
        V = split(X @ Wv + bv)
        scale = HD ** -0.25
        m4 = mask[:, None, :, None]
        Qs, Ks, Vs = Q * scale, K * scale * m4, V * m4
        ratio = M ** -0.5

        def feat(x, is_q):
            x = x * (HD ** -0.25)
            u = jnp.einsum('bhsd,md->bhsm', x, proj)
            diag = 0.5 * jnp.sum(x * x, axis=-1, keepdims=True)
            stab = (jnp.max(u, axis=-1, keepdims=True) if is_q
                    else jnp.max(u, axis=(-1, -2), keepdims=True))
            return ratio * (jnp.exp(u - diag - stab) + 1e-4)

        qp, kp = feat(Qs, True), feat(Ks, False)
        kv = jnp.einsum('bhsm,bhsd->bhmd', kp, Vs)
        z = 1.0 / (jnp.einsum('bhsm,bhm->bhs', qp, jnp.sum(kp, axis=2)) + 1e-6)
        out = jnp.einsum('bhsm,bhmd->bhsd', qp, kv) * z[..., None]
        return out.transpose(0, 2, 1, 3).reshape(B, S, D)

    jfn = jax.jit(compute,
                  in_shardings=(seq, rep, col, vec, col, vec, col, vec, rep),
                  out_shardings=outsh)
    args = [jax.device_put(np.asarray(a, np.float32), s) for a, s in
            zip((X, mask, Wq, bq, Wk, bk, Wv, bv, proj),
                (seq, rep, col, vec, col, vec, col, vec, rep))]
    return np.asarray(jfn(*args), dtype=np.float32)


def kernel(X, mask, Wq, bq, Wk, bk, Wv, bv, proj):
    global _use_fast, _fleet
    inputs = {"X": X, "mask": mask, "Wq": Wq, "bq": bq, "Wk": Wk, "bk": bk,
              "Wv": Wv, "bv": bv, "proj": proj}
    with _lock:
        if _use_fast:
            try:
                return _run_fast(inputs)
            except Exception:
                _use_fast = False
                if _fleet is not None:
                    _fleet.close()
                    _fleet = None
        return _run_fallback(**inputs)
